# revision 24
# baseline (speedup 1.0000x reference)
"""CP-factorized embedding lookup on 8 TRN2 NeuronCores.

Reference computes full[a,b,c,d,e,f] = sum_r U0[a,r]*...*U5[f,r], reshapes to a
(50000, 512) table, and gathers rows by x. We never materialize the table:

  out[n, e] = sum_r (U0[a_n,r]*U1[b_n,r]*U2[c_n,r]) * (U3[d,r]*U4[e2,r]*U5[f,r])
            = sum_r V[n, r] * W[e, r]

with v = 1000a + 25b + c and e = 64d + 8e2 + f.

Input staging is packet-count optimized (DMA engines are bound by ~100ns per
partition-packet, not bytes): x lands as ONE 4KB packet and is replicated
across the 115 factor-row partitions by a GpSimd partition_broadcast; all
constants ship TRANSPOSED in a 32-partition aux tensor (32 packets) and are
restored on-chip through PE transposes.  The block-diagonal [U0;U1;U2] gather
operand is rebuilt from its compact [32, 115] transpose with one iota +
scalar_tensor_tensor (mask-multiply) op.

Per core (1024 indices, data-parallel over the 8192 total), two pipelined
512-index halves:
  1. decompose+one-hot in a fused 3+1-op DVE chain per half with per-partition
     constants (biases make the f32->i16 round-to-nearest casts exact floors):
       s1'[p,n] = rint(v*c1m[p] + c1a[p]) ; s2[p,n] = rint(v*c2m[p] + c2a[p])
       (rows 0:50: s2 = min(v,1) -- folds the v==0 padding mask in)
       onehot   = is_equal(s2*K[p], s1')   (bf16 out)
  2. gather via one PE matmul per half (bf16) -> psum[96, 512]; the Scalar
     engine stages psum to SBUF and GpSimd forms V = block0*block1*block2
  3. W[32, 512] = Khatri-Rao of U3,U4,U5 built on GpSimd (U345 transposed
     via the DVE 32x32 stream transpose)
  4. output matmuls (bf16): per 256-row group, two matmuls with even/odd
     interleaved vth columns -> psum[128, 1024] so partition p holds rows
     {r0+2p, r0+2p+1}; engine copy casts f32->bf16 and one DMA per group
     writes 128 contiguous 2KB packets.

All aux content is pure host-side rearrangement/zero-padding of inputs --
every arithmetic op stays on device.  Matmul operands are bf16; one-hot
entries are exact in bf16 and factor rounding is ~1e-2 relative worst-case,
inside the 2e-2 tolerance.  The output travels as bf16 (halves the DMA) and
is widened to f32 on the host.
"""

import numpy as np

import concourse.bass as bass
import concourse.mybir as mybir
import concourse.tile as tile
from concourse import bacc
from concourse.bass_utils import run_bass_kernel_spmd

F32 = mybir.dt.float32
I32 = mybir.dt.int32
I16 = mybir.dt.int16
U16 = mybir.dt.uint16
BF16 = mybir.dt.bfloat16
ALU = mybir.AluOpType

N_CORES = 8
PER_CORE = 1024           # indices per core (8192 / 8)
HALF = 512                # pipeline granularity
EMB = 512
RANK = 32
KV = 115                  # 50 + 40 + 25 stacked vocab-factor rows

R1000 = float(np.float32(1.0 / 1000.0))
R25 = float(np.float32(1.0 / 25.0))

# auxT layout: [32, W] f32 (transposed constants; 32 DMA packets)
CCT_OFF = 0      # rows 0:6, cols 0:115: ccT (c1m, c1a, c2m, c2a, K, blockid)
EYE_OFF = 115    # cols 115:147: eye(32) (top-left 6x6 doubles as eye(6))
U345_OFF = 147   # cols 147:179: [U3;U4;U5] rows 0:24, zero-padded to 32
U012T_OFF = 179  # cols 179:294: [U0;U1;U2] block-stacked, transposed [32,115]
AUX_W = 294

MM_DT = BF16


def _cct_table() -> np.ndarray:
    """[6, 115] per-partition constants (transposed).

    s1' = rint(v*c1m + c1a);  s2 = rint(v*c2m + c2a)  (rows 0:50: min(v,1))
    onehot[p] = (K[p]*s2 == s1')
    Integer offsets commute with round-to-nearest, so folding the one-hot
    row offset into c1a keeps the floors exact.
    """
    cc = np.zeros((6, KV), np.float32)
    p = np.arange(KV, dtype=np.float32)
    # block 0 (rows 0:50): s1' = floor(v/1000) + 1000 - p ; s2 = min(v,1)
    cc[0, 0:50] = R1000
    cc[1, 0:50] = np.float32(-499.5 * R1000) + 1000.0 - p[0:50]
    cc[4, 0:50] = 1000.0
    # block 1 (rows 50:90): s1' = floor(v/25) + 50 - p ; s2 = floor(v/1000)
    cc[0, 50:90] = R25
    cc[1, 50:90] = np.float32(-12.0 * R25) + 50.0 - p[50:90]
    cc[2, 50:90] = R1000
    cc[3, 50:90] = np.float32(-499.5 * R1000)
    cc[4, 50:90] = 40.0
    cc[5, 50:90] = 1.0
    # block 2 (rows 90:115): s1' = v - 25000 + 90 - p ; s2 = floor(v/25) - 1000
    cc[0, 90:115] = 1.0
    cc[1, 90:115] = -25000.0 + 90.0 - p[90:115]
    cc[2, 90:115] = R25
    cc[3, 90:115] = np.float32(-25012.0 * R25)
    cc[4, 90:115] = 25.0
    cc[5, 90:115] = 2.0
    return cc


def _aux_table(us: list[np.ndarray]) -> np.ndarray:
    aux = np.zeros((32, AUX_W), np.float32)
    aux[0:6, CCT_OFF:CCT_OFF + KV] = _cct_table()
    aux[:, EYE_OFF:EYE_OFF + 32] = np.eye(32, dtype=np.float32)
    # u345 rows are the factor matrices stacked (24 x 32), zero-padded to 32
    aux[0:8, U345_OFF:U345_OFF + 32] = us[3]
    aux[8:16, U345_OFF:U345_OFF + 32] = us[4]
    aux[16:24, U345_OFF:U345_OFF + 32] = us[5]
    # U012T[r, p]: block-stacked factor rows, transposed
    u012 = np.zeros((KV, RANK), np.float32)
    u012[0:50] = us[0]
    u012[50:90] = us[1]
    u012[90:115] = us[2]
    aux[:, U012T_OFF:U012T_OFF + KV] = u012.T
    return aux


def build():
    nc = bacc.Bacc("TRN2", target_bir_lowering=False, debug=False)

    # x int32 narrowed to u16 host-side (pure byte selection; values < 2^16),
    # replicated 4x so the partition-broadcast DMAs read distinct sources and
    # engage more DMA engines (a single-source broadcast pins to ~5 of 16).
    x = nc.dram_tensor("x", [4, PER_CORE], U16, kind="ExternalInput")
    aux_d = nc.dram_tensor("aux", [32, AUX_W], F32, kind="ExternalInput")
    out = nc.dram_tensor("out", [PER_CORE, EMB], BF16, kind="ExternalOutput")

    NH = PER_CORE // HALF   # 2 halves
    NG = PER_CORE // 256    # 4 output groups of 256 rows

    with tile.TileContext(nc) as tc:
        with (
            tc.tile_pool(name="const", bufs=1) as cpool,
            tc.tile_pool(name="work", bufs=2) as wpool,
            # bufs=1: half B's writes must wait for half A's readers, which
            # keeps the scheduler from starving half A's downstream ops
            tc.tile_pool(name="ser", bufs=1) as spool,
            tc.tile_pool(name="gps", bufs=2, space="PSUM") as gpsum,
            tc.tile_pool(name="tps", bufs=1, space="PSUM") as tpsum,
            tc.tile_pool(name="ops", bufs=2, space="PSUM") as opsum,
            tc.tile_pool(name="osb", bufs=4) as opool,
        ):
            # ---- input DMAs: the x broadcast is split into 4 row-blocks from
            # 4 replicas across both HWDGE queues (115 2KB packets total);
            # auxT is 32 packets on the scalar queue.
            xrep = cpool.tile([KV, PER_CORE], U16)
            splits = [(0, 58), (58, KV)]
            for k, (lo, hi) in enumerate(splits):
                eng = nc.sync if k % 2 == 0 else nc.scalar
                eng.dma_start(
                    out=xrep[lo:hi, :],
                    in_=x[k].unsqueeze(0).partition_broadcast(hi - lo),
                )
            auxs = cpool.tile([32, AUX_W], F32)
            nc.scalar.dma_start(out=auxs[:], in_=aux_d[:])

            eye32 = auxs[:, EYE_OFF:EYE_OFF + 32]

            # iota block-ids [115, 3x32] (no input deps; fires immediately)
            iot = cpool.tile([KV, 3 * RANK], I16)
            nc.gpsimd.iota(
                out=iot[:].rearrange("p (b c) -> p b c", b=3),
                pattern=[[1, 3], [0, RANK]], base=0, channel_multiplier=0,
            )

            # ---- restore per-partition constants via PE transposes
            cc_ps = tpsum.tile([KV, 6], F32)
            nc.tensor.transpose(
                cc_ps[:], auxs[0:6, CCT_OFF:CCT_OFF + KV], eye32[0:6, 0:6]
            )
            cc = cpool.tile([KV, 6], F32)
            nc.scalar.copy(out=cc[:], in_=cc_ps[:])

            u012_ps = tpsum.tile([KV, RANK], F32)
            nc.tensor.transpose(
                u012_ps[:], auxs[:, U012T_OFF:U012T_OFF + KV], eye32
            )
            # block-diag [U0;U1;U2] in bf16: (blockid == iota_b) * U012
            ublk = cpool.tile([KV, 3 * RANK], MM_DT)
            nc.vector.scalar_tensor_tensor(
                out=ublk[:].rearrange("p (b c) -> p b c", b=3),
                in0=iot[:].rearrange("p (b c) -> p b c", b=3),
                scalar=cc[:, 5:6],
                in1=u012_ps[:].unsqueeze(1).broadcast_to([KV, 3, RANK]),
                op0=ALU.is_equal, op1=ALU.mult,
            )

            # u345t via the DVE 32x32 stream transpose (no PSUM round-trip)
            u345t = cpool.tile([RANK, 32], F32)
            nc.vector.transpose(
                out=u345t[:], in_=auxs[:, U345_OFF:U345_OFF + 32]
            )

            # ---- W[r, e] = U3[d,r] * U4[e2,r] * U5[f,r] on GpSimd
            t45 = cpool.tile([RANK, 64], F32)
            nc.gpsimd.tensor_tensor(
                out=t45[:].rearrange("r (e f) -> r e f", e=8),
                in0=u345t[:, 8:16].unsqueeze(2).broadcast_to([RANK, 8, 8]),
                in1=u345t[:, 16:24].unsqueeze(1).broadcast_to([RANK, 8, 8]),
                op=ALU.mult,
            )
            wt = cpool.tile([RANK, EMB], MM_DT)
            nc.gpsimd.tensor_tensor(
                out=wt[:].rearrange("r (d ef) -> r d ef", d=8),
                in0=u345t[:, 0:8].unsqueeze(2).broadcast_to([RANK, 8, 64]),
                in1=t45[:].unsqueeze(1).broadcast_to([RANK, 8, 64]),
                op=ALU.mult,
            )

            vth = [None] * NH
            for h in range(NH):
                xh = xrep[:, h * HALF:(h + 1) * HALF]
                # fused decomposition: 3 tensor_scalar + 1 scalar_tensor_tensor
                s1 = spool.tile([KV, HALF], I16, name=f"s1_{h}", tag="s1")
                nc.vector.tensor_scalar(
                    out=s1[:], in0=xh, scalar1=cc[:, 0:1], scalar2=cc[:, 1:2],
                    op0=ALU.mult, op1=ALU.add,
                )
                s2 = spool.tile([KV, HALF], I16, name=f"s2_{h}", tag="s2")
                nc.vector.tensor_scalar(
                    out=s2[:], in0=xh, scalar1=cc[:, 2:3], scalar2=cc[:, 3:4],
                    op0=ALU.mult, op1=ALU.add,
                )
                # rows 0:50: s2 = min(v, 1) folds the v==0 padding mask into
                # the block-0 one-hot (partition starts must be 32-aligned,
                # so overwrite rather than split at row 50)
                nc.vector.tensor_scalar(
                    out=s2[0:50, :], in0=xh[0:50, :], scalar1=1.0, scalar2=1.0,
                    op0=ALU.min, op1=ALU.mult,
                )
                onehot = wpool.tile([KV, HALF], MM_DT, name=f"oh_{h}", tag="oh")
                nc.vector.scalar_tensor_tensor(
                    out=onehot[:], in0=s2[:], scalar=cc[:, 4:5], in1=s1[:],
                    op0=ALU.mult, op1=ALU.is_equal,
                )

                pv = gpsum.tile([3 * RANK, HALF], F32, name=f"pv_{h}", tag="pv")
                nc.tensor.matmul(
                    pv[:], lhsT=ublk[:], rhs=onehot[:], start=True, stop=True,
                )
                # Two SBUF inputs must share a base partition, PSUM inputs
                # are exempt: stage block 0 to SBUF on the Scalar engine,
                # keep blocks 1/2 as the PSUM operand of each multiply.
                s0 = spool.tile([RANK, HALF], F32, name=f"s0_{h}", tag="s0")
                nc.scalar.copy(out=s0[:], in_=pv[0:32, :])
                v01 = spool.tile([RANK, HALF], F32, name=f"v01_{h}", tag="v01")
                nc.vector.tensor_tensor(
                    out=v01[:], in0=s0[:], in1=pv[32:64, :], op=ALU.mult
                )
                vth[h] = cpool.tile([RANK, HALF], MM_DT, name=f"vt_{h}")
                nc.vector.tensor_tensor(
                    out=vth[h][:], in0=v01[:], in1=pv[64:96, :], op=ALU.mult
                )

            # ---- output: per 256-row group, even/odd interleaved columns so
            # partition p holds DRAM rows {r0+2p, r0+2p+1} -> one contiguous
            # 2KB packet per partition per group.  The last group's cast and
            # DMA are split across both engines/partition halves to shorten
            # the tail.
            for g in range(NG):
                h, lo = g // 2, (g % 2) * 256
                vsel = vth[h][:].rearrange("r (n two) -> r n two", two=2)
                po = opsum.tile([128, 2 * EMB], F32, name=f"po_{g}", tag="po")
                for j in range(2):
                    nc.tensor.matmul(
                        po[:, j * EMB:(j + 1) * EMB],
                        lhsT=vsel[:, lo // 2:lo // 2 + 128, j],
                        rhs=wt[:], start=True, stop=True,
                    )
                osb = opool.tile([128, 2 * EMB], MM_DT, name=f"osb_{g}",
                                 tag="osb")
                row0 = g * 256
                oview = out[row0:row0 + 256, :].rearrange(
                    "(p j) e -> p (j e)", j=2
                )
                # cast column halves on both engines in parallel (cost is
                # free-dim bound, so this genuinely halves the copy latency)
                e0 = nc.scalar if g % 2 == 0 else nc.vector
                e1 = nc.vector if g % 2 == 0 else nc.scalar
                if e0 is nc.scalar:
                    nc.scalar.copy(out=osb[:, 0:EMB], in_=po[:, 0:EMB])
                    nc.vector.tensor_copy(
                        out=osb[:, EMB:2 * EMB], in_=po[:, EMB:2 * EMB]
                    )
                else:
                    nc.vector.tensor_copy(out=osb[:, 0:EMB], in_=po[:, 0:EMB])
                    nc.scalar.copy(
                        out=osb[:, EMB:2 * EMB], in_=po[:, EMB:2 * EMB]
                    )
                nc.sync.dma_start(out=oview, in_=osb[:])

    nc.compile()
    return nc


_CACHE: dict = {}


def _get_nc():
    if "nc" not in _CACHE:
        _CACHE["nc"] = build()
    return _CACHE["nc"]


def run(inputs, **spmd_kwargs):
    nc = _get_nc()
    x = np.ascontiguousarray(inputs["x"].reshape(-1), dtype=np.int32)
    # little-endian u16 view, even offsets hold the values (pure byte pick)
    xu = np.ascontiguousarray(x.view(np.uint16)[0::2])
    us = [
        np.ascontiguousarray(inputs[f"U{j}"], dtype=np.float32) for j in range(6)
    ]
    aux = _aux_table(us)
    in_maps = []
    for i in range(N_CORES):
        xc = xu[i * PER_CORE:(i + 1) * PER_CORE]
        x4 = np.ascontiguousarray(np.broadcast_to(xc, (4, PER_CORE)))
        in_maps.append({"x": x4, "aux": aux})
    res = run_bass_kernel_spmd(
        nc, in_maps, core_ids=list(range(N_CORES)), **spmd_kwargs
    )
    shards = [
        np.asarray(res.results[i]["out"]).astype(np.float32)
        for i in range(N_CORES)
    ]
    full = np.concatenate(shards, axis=0).reshape(4, 2048, EMB)
    return full, res


def kernel(**inputs) -> np.ndarray:
    return run(inputs)[0]


# revision 25
# speedup vs baseline: 1.0779x; 1.0779x over previous
"""CP-factorized embedding lookup on 8 TRN2 NeuronCores.

Reference computes full[a,b,c,d,e,f] = sum_r U0[a,r]*...*U5[f,r], reshapes to a
(50000, 512) table, and gathers rows by x. We never materialize the table:

  out[n, e] = sum_r (U0[a_n,r]*U1[b_n,r]*U2[c_n,r]) * (U3[d,r]*U4[e2,r]*U5[f,r])
            = sum_r V[n, r] * W[e, r]

with v = 1000a + 25b + c and e = 64d + 8e2 + f.

Input staging is packet-count optimized (DMA engines are bound by ~100ns per
partition-packet, not bytes): x lands as ONE 4KB packet and is replicated
across the 115 factor-row partitions by a GpSimd partition_broadcast; all
constants ship TRANSPOSED in a 32-partition aux tensor (32 packets) and are
restored on-chip through PE transposes.  The block-diagonal [U0;U1;U2] gather
operand is rebuilt from its compact [32, 115] transpose with one iota +
scalar_tensor_tensor (mask-multiply) op.

Per core (1024 indices, data-parallel over the 8192 total), two pipelined
512-index halves:
  1. decompose+one-hot in a fused 3+1-op DVE chain per half with per-partition
     constants (biases make the f32->i16 round-to-nearest casts exact floors):
       s1'[p,n] = rint(v*c1m[p] + c1a[p]) ; s2[p,n] = rint(v*c2m[p] + c2a[p])
       (rows 0:50: s2 = min(v,1) -- folds the v==0 padding mask in)
       onehot   = is_equal(s2*K[p], s1')   (bf16 out)
  2. gather via one PE matmul per half (bf16) -> psum[96, 512]; the Scalar
     engine stages psum to SBUF and GpSimd forms V = block0*block1*block2
  3. W[32, 512] = Khatri-Rao of U3,U4,U5 built on GpSimd (U345 transposed
     via the DVE 32x32 stream transpose)
  4. output matmuls (bf16): per 256-row group, two matmuls with even/odd
     interleaved vth columns -> psum[128, 1024] so partition p holds rows
     {r0+2p, r0+2p+1}; engine copy casts f32->bf16 and one DMA per group
     writes 128 contiguous 2KB packets.

All aux content is pure host-side rearrangement/zero-padding of inputs --
every arithmetic op stays on device.  Matmul operands are bf16; one-hot
entries are exact in bf16 and factor rounding is ~1e-2 relative worst-case,
inside the 2e-2 tolerance.  The output travels as bf16 (halves the DMA) and
is widened to f32 on the host.
"""

import numpy as np

import concourse.bass as bass
import concourse.mybir as mybir
import concourse.tile as tile
from concourse import bacc
from concourse.bass_utils import run_bass_kernel_spmd

F32 = mybir.dt.float32
I32 = mybir.dt.int32
I16 = mybir.dt.int16
U16 = mybir.dt.uint16
BF16 = mybir.dt.bfloat16
ALU = mybir.AluOpType

N_CORES = 8
PER_CORE = 1024           # indices per core (8192 / 8)
HALF = 512                # pipeline granularity
EMB = 512
RANK = 32
KV = 115                  # 50 + 40 + 25 stacked vocab-factor rows

R1000 = float(np.float32(1.0 / 1000.0))
R25 = float(np.float32(1.0 / 25.0))

# auxT layout: [32, W] f32 (transposed constants; 32 DMA packets)
CCT_OFF = 0      # rows 0:6, cols 0:115: ccT (c1m, c1a, c2m, c2a, K, blockid)
EYE_OFF = 115    # cols 115:147: eye(32) (top-left 6x6 doubles as eye(6))
U345_OFF = 147   # cols 147:179: [U3;U4;U5] rows 0:24, zero-padded to 32
U012T_OFF = 179  # cols 179:294: [U0;U1;U2] block-stacked, transposed [32,115]
AUX_W = 294

MM_DT = BF16


def _cct_table() -> np.ndarray:
    """[6, 115] per-partition constants (transposed).

    s1' = rint(v*c1m + c1a);  s2 = rint(v*c2m + c2a)  (rows 0:50: min(v,1))
    onehot[p] = (K[p]*s2 == s1')
    Integer offsets commute with round-to-nearest, so folding the one-hot
    row offset into c1a keeps the floors exact.
    """
    cc = np.zeros((6, KV), np.float32)
    p = np.arange(KV, dtype=np.float32)
    # block 0 (rows 0:50): s1' = floor(v/1000) + 1000 - p ; s2 = min(v,1)
    cc[0, 0:50] = R1000
    cc[1, 0:50] = np.float32(-499.5 * R1000) + 1000.0 - p[0:50]
    cc[4, 0:50] = 1000.0
    # block 1 (rows 50:90): s1' = floor(v/25) + 50 - p ; s2 = floor(v/1000)
    cc[0, 50:90] = R25
    cc[1, 50:90] = np.float32(-12.0 * R25) + 50.0 - p[50:90]
    cc[2, 50:90] = R1000
    cc[3, 50:90] = np.float32(-499.5 * R1000)
    cc[4, 50:90] = 40.0
    cc[5, 50:90] = 1.0
    # block 2 (rows 90:115): s1' = v - 25000 + 90 - p ; s2 = floor(v/25) - 1000
    cc[0, 90:115] = 1.0
    cc[1, 90:115] = -25000.0 + 90.0 - p[90:115]
    cc[2, 90:115] = R25
    cc[3, 90:115] = np.float32(-25012.0 * R25)
    cc[4, 90:115] = 25.0
    cc[5, 90:115] = 2.0
    return cc


def _aux_table(us: list[np.ndarray]) -> np.ndarray:
    aux = np.zeros((32, AUX_W), np.float32)
    aux[0:6, CCT_OFF:CCT_OFF + KV] = _cct_table()
    aux[:, EYE_OFF:EYE_OFF + 32] = np.eye(32, dtype=np.float32)
    # u345 rows are the factor matrices stacked (24 x 32), zero-padded to 32
    aux[0:8, U345_OFF:U345_OFF + 32] = us[3]
    aux[8:16, U345_OFF:U345_OFF + 32] = us[4]
    aux[16:24, U345_OFF:U345_OFF + 32] = us[5]
    # U012T[r, p]: block-stacked factor rows, transposed
    u012 = np.zeros((KV, RANK), np.float32)
    u012[0:50] = us[0]
    u012[50:90] = us[1]
    u012[90:115] = us[2]
    aux[:, U012T_OFF:U012T_OFF + KV] = u012.T
    return aux


def build():
    nc = bacc.Bacc("TRN2", target_bir_lowering=False, debug=False)

    # x int32 narrowed to u16 host-side (pure byte selection; values < 2^16),
    # replicated 4x so the partition-broadcast DMAs read distinct sources and
    # engage more DMA engines (a single-source broadcast pins to ~5 of 16).
    x = nc.dram_tensor("x", [4, PER_CORE], U16, kind="ExternalInput")
    aux_d = nc.dram_tensor("aux", [32, AUX_W], F32, kind="ExternalInput")
    out = nc.dram_tensor("out", [PER_CORE, EMB], BF16, kind="ExternalOutput")

    NH = PER_CORE // HALF   # 2 halves
    NG = PER_CORE // 256    # 4 output groups of 256 rows

    with tile.TileContext(nc) as tc:
        with (
            tc.tile_pool(name="const", bufs=1) as cpool,
            tc.tile_pool(name="work", bufs=2) as wpool,
            # bufs=1: half B's writes must wait for half A's readers, which
            # keeps the scheduler from starving half A's downstream ops
            tc.tile_pool(name="ser", bufs=1) as spool,
            tc.tile_pool(name="gps", bufs=2, space="PSUM") as gpsum,
            tc.tile_pool(name="tps", bufs=1, space="PSUM") as tpsum,
            tc.tile_pool(name="ops", bufs=2, space="PSUM") as opsum,
            tc.tile_pool(name="osb", bufs=4) as opool,
        ):
            # ---- input DMAs: the x broadcast is split into 4 row-blocks from
            # 4 replicas across both HWDGE queues (115 2KB packets total);
            # auxT is 32 packets on the scalar queue.
            xrep = cpool.tile([KV, PER_CORE], U16)
            splits = [(0, 32), (32, 64), (64, 96), (96, KV)]
            for k, (lo, hi) in enumerate(splits):
                eng = nc.sync if k % 2 == 0 else nc.scalar
                eng.dma_start(
                    out=xrep[lo:hi, :],
                    in_=x[k].unsqueeze(0).partition_broadcast(hi - lo),
                )
            auxs = cpool.tile([32, AUX_W], F32)
            nc.scalar.dma_start(out=auxs[:], in_=aux_d[:])

            eye32 = auxs[:, EYE_OFF:EYE_OFF + 32]

            # iota block-ids [115, 3x32] (no input deps; fires immediately)
            iot = cpool.tile([KV, 3 * RANK], I16)
            nc.gpsimd.iota(
                out=iot[:].rearrange("p (b c) -> p b c", b=3),
                pattern=[[1, 3], [0, RANK]], base=0, channel_multiplier=0,
            )

            # ---- restore per-partition constants via PE transposes
            cc_ps = tpsum.tile([KV, 6], F32)
            nc.tensor.transpose(
                cc_ps[:], auxs[0:6, CCT_OFF:CCT_OFF + KV], eye32[0:6, 0:6]
            )
            cc = cpool.tile([KV, 6], F32)
            nc.scalar.copy(out=cc[:], in_=cc_ps[:])

            u012_ps = tpsum.tile([KV, RANK], F32)
            nc.tensor.transpose(
                u012_ps[:], auxs[:, U012T_OFF:U012T_OFF + KV], eye32
            )
            # block-diag [U0;U1;U2] in bf16: (blockid == iota_b) * U012
            ublk = cpool.tile([KV, 3 * RANK], MM_DT)
            nc.vector.scalar_tensor_tensor(
                out=ublk[:].rearrange("p (b c) -> p b c", b=3),
                in0=iot[:].rearrange("p (b c) -> p b c", b=3),
                scalar=cc[:, 5:6],
                in1=u012_ps[:].unsqueeze(1).broadcast_to([KV, 3, RANK]),
                op0=ALU.is_equal, op1=ALU.mult,
            )

            # u345t via the DVE 32x32 stream transpose (no PSUM round-trip)
            u345t = cpool.tile([RANK, 32], F32)
            nc.vector.transpose(
                out=u345t[:], in_=auxs[:, U345_OFF:U345_OFF + 32]
            )

            # ---- W[r, e] = U3[d,r] * U4[e2,r] * U5[f,r] on GpSimd
            t45 = cpool.tile([RANK, 64], F32)
            nc.gpsimd.tensor_tensor(
                out=t45[:].rearrange("r (e f) -> r e f", e=8),
                in0=u345t[:, 8:16].unsqueeze(2).broadcast_to([RANK, 8, 8]),
                in1=u345t[:, 16:24].unsqueeze(1).broadcast_to([RANK, 8, 8]),
                op=ALU.mult,
            )
            wt = cpool.tile([RANK, EMB], MM_DT)
            nc.gpsimd.tensor_tensor(
                out=wt[:].rearrange("r (d ef) -> r d ef", d=8),
                in0=u345t[:, 0:8].unsqueeze(2).broadcast_to([RANK, 8, 64]),
                in1=t45[:].unsqueeze(1).broadcast_to([RANK, 8, 64]),
                op=ALU.mult,
            )

            vth = [None] * NH
            for h in range(NH):
                xh = xrep[:, h * HALF:(h + 1) * HALF]
                # fused decomposition: 3 tensor_scalar + 1 scalar_tensor_tensor
                s1 = wpool.tile([KV, HALF], I16, name=f"s1_{h}", tag="s1")
                nc.vector.tensor_scalar(
                    out=s1[:], in0=xh, scalar1=cc[:, 0:1], scalar2=cc[:, 1:2],
                    op0=ALU.mult, op1=ALU.add,
                )
                s2 = wpool.tile([KV, HALF], I16, name=f"s2_{h}", tag="s2")
                nc.vector.tensor_scalar(
                    out=s2[:], in0=xh, scalar1=cc[:, 2:3], scalar2=cc[:, 3:4],
                    op0=ALU.mult, op1=ALU.add,
                )
                # rows 0:50: s2 = min(v, 1) folds the v==0 padding mask into
                # the block-0 one-hot (partition starts must be 32-aligned,
                # so overwrite rather than split at row 50)
                nc.vector.tensor_scalar(
                    out=s2[0:50, :], in0=xh[0:50, :], scalar1=1.0, scalar2=1.0,
                    op0=ALU.min, op1=ALU.mult,
                )
                onehot = wpool.tile([KV, HALF], MM_DT, name=f"oh_{h}", tag="oh")
                nc.vector.scalar_tensor_tensor(
                    out=onehot[:], in0=s2[:], scalar=cc[:, 4:5], in1=s1[:],
                    op0=ALU.mult, op1=ALU.is_equal,
                )

                pv = gpsum.tile([3 * RANK, HALF], F32, name=f"pv_{h}", tag="pv")
                nc.tensor.matmul(
                    pv[:], lhsT=ublk[:], rhs=onehot[:], start=True, stop=True,
                )
                # Two SBUF inputs must share a base partition, PSUM inputs
                # are exempt: stage block 0 to SBUF on the Scalar engine,
                # keep blocks 1/2 as the PSUM operand of each multiply.
                s0 = spool.tile([RANK, HALF], F32, name=f"s0_{h}", tag="s0")
                nc.scalar.copy(out=s0[:], in_=pv[0:32, :])
                v01 = spool.tile([RANK, HALF], F32, name=f"v01_{h}", tag="v01")
                nc.vector.tensor_tensor(
                    out=v01[:], in0=s0[:], in1=pv[32:64, :], op=ALU.mult
                )
                vth[h] = cpool.tile([RANK, HALF], MM_DT, name=f"vt_{h}")
                nc.vector.tensor_tensor(
                    out=vth[h][:], in0=v01[:], in1=pv[64:96, :], op=ALU.mult
                )

            # ---- output: per 256-row group, even/odd interleaved columns so
            # partition p holds DRAM rows {r0+2p, r0+2p+1} -> one contiguous
            # 2KB packet per partition per group.  The last group's cast and
            # DMA are split across both engines/partition halves to shorten
            # the tail.
            for g in range(NG):
                h, lo = g // 2, (g % 2) * 256
                vsel = vth[h][:].rearrange("r (n two) -> r n two", two=2)
                po = opsum.tile([128, 2 * EMB], F32, name=f"po_{g}", tag="po")
                for j in range(2):
                    nc.tensor.matmul(
                        po[:, j * EMB:(j + 1) * EMB],
                        lhsT=vsel[:, lo // 2:lo // 2 + 128, j],
                        rhs=wt[:], start=True, stop=True,
                    )
                osb = opool.tile([128, 2 * EMB], MM_DT, name=f"osb_{g}",
                                 tag="osb")
                row0 = g * 256
                oview = out[row0:row0 + 256, :].rearrange(
                    "(p j) e -> p (j e)", j=2
                )
                # cast column halves on both engines in parallel (cost is
                # free-dim bound, so this genuinely halves the copy latency)
                e0 = nc.scalar if g % 2 == 0 else nc.vector
                e1 = nc.vector if g % 2 == 0 else nc.scalar
                if e0 is nc.scalar:
                    nc.scalar.copy(out=osb[:, 0:EMB], in_=po[:, 0:EMB])
                    nc.vector.tensor_copy(
                        out=osb[:, EMB:2 * EMB], in_=po[:, EMB:2 * EMB]
                    )
                else:
                    nc.vector.tensor_copy(out=osb[:, 0:EMB], in_=po[:, 0:EMB])
                    nc.scalar.copy(
                        out=osb[:, EMB:2 * EMB], in_=po[:, EMB:2 * EMB]
                    )
                nc.sync.dma_start(out=oview, in_=osb[:])

    nc.compile()
    return nc


_CACHE: dict = {}


def _get_nc():
    if "nc" not in _CACHE:
        _CACHE["nc"] = build()
    return _CACHE["nc"]


def run(inputs, **spmd_kwargs):
    nc = _get_nc()
    x = np.ascontiguousarray(inputs["x"].reshape(-1), dtype=np.int32)
    # little-endian u16 view, even offsets hold the values (pure byte pick)
    xu = np.ascontiguousarray(x.view(np.uint16)[0::2])
    us = [
        np.ascontiguousarray(inputs[f"U{j}"], dtype=np.float32) for j in range(6)
    ]
    aux = _aux_table(us)
    in_maps = []
    for i in range(N_CORES):
        xc = xu[i * PER_CORE:(i + 1) * PER_CORE]
        x4 = np.ascontiguousarray(np.broadcast_to(xc, (4, PER_CORE)))
        in_maps.append({"x": x4, "aux": aux})
    res = run_bass_kernel_spmd(
        nc, in_maps, core_ids=list(range(N_CORES)), **spmd_kwargs
    )
    shards = [
        np.asarray(res.results[i]["out"]).astype(np.float32)
        for i in range(N_CORES)
    ]
    full = np.concatenate(shards, axis=0).reshape(4, 2048, EMB)
    return full, res


def kernel(**inputs) -> np.ndarray:
    return run(inputs)[0]


# revision 26
# speedup vs baseline: 1.2316x; 1.1426x over previous
"""CP-factorized embedding lookup on 8 TRN2 NeuronCores.

Reference computes full[a,b,c,d,e,f] = sum_r U0[a,r]*...*U5[f,r], reshapes to a
(50000, 512) table, and gathers rows by x. We never materialize the table:

  out[n, e] = sum_r (U0[a_n,r]*U1[b_n,r]*U2[c_n,r]) * (U3[d,r]*U4[e2,r]*U5[f,r])
            = sum_r V[n, r] * W[e, r]

with v = 1000a + 25b + c and e = 64d + 8e2 + f.

Input staging is packet-count optimized (DMA engines are bound by ~100ns per
partition-packet, not bytes): x lands as ONE 4KB packet and is replicated
across the 115 factor-row partitions by a GpSimd partition_broadcast; all
constants ship TRANSPOSED in a 32-partition aux tensor (32 packets) and are
restored on-chip through PE transposes.  The block-diagonal [U0;U1;U2] gather
operand is rebuilt from its compact [32, 115] transpose with one iota +
scalar_tensor_tensor (mask-multiply) op.

Per core (1024 indices, data-parallel over the 8192 total), two pipelined
512-index halves:
  1. decompose+one-hot in a fused 3+1-op DVE chain per half with per-partition
     constants (biases make the f32->i16 round-to-nearest casts exact floors):
       s1'[p,n] = rint(v*c1m[p] + c1a[p]) ; s2[p,n] = rint(v*c2m[p] + c2a[p])
       (rows 0:50: s2 = min(v,1) -- folds the v==0 padding mask in)
       onehot   = is_equal(s2*K[p], s1')   (bf16 out)
  2. gather via one PE matmul per half (bf16) -> psum[96, 512]; the Scalar
     engine stages psum to SBUF and GpSimd forms V = block0*block1*block2
  3. W[32, 512] = Khatri-Rao of U3,U4,U5 built on GpSimd (U345 transposed
     via the DVE 32x32 stream transpose)
  4. output matmuls (bf16): per 256-row group, two matmuls with even/odd
     interleaved vth columns -> psum[128, 1024] so partition p holds rows
     {r0+2p, r0+2p+1}; engine copy casts f32->bf16 and one DMA per group
     writes 128 contiguous 2KB packets.

All aux content is pure host-side rearrangement/zero-padding of inputs --
every arithmetic op stays on device.  Matmul operands are bf16; one-hot
entries are exact in bf16 and factor rounding is ~1e-2 relative worst-case,
inside the 2e-2 tolerance.  The output travels as bf16 (halves the DMA) and
is widened to f32 on the host.
"""

import numpy as np

import concourse.bass as bass
import concourse.mybir as mybir
import concourse.tile as tile
from concourse import bacc
from concourse.bass_utils import run_bass_kernel_spmd

F32 = mybir.dt.float32
I32 = mybir.dt.int32
I16 = mybir.dt.int16
U16 = mybir.dt.uint16
BF16 = mybir.dt.bfloat16
ALU = mybir.AluOpType

N_CORES = 8
PER_CORE = 1024           # indices per core (8192 / 8)
HALF = 512                # pipeline granularity
EMB = 512
RANK = 32
KV = 115                  # 50 + 40 + 25 stacked vocab-factor rows

R1000 = float(np.float32(1.0 / 1000.0))
R25 = float(np.float32(1.0 / 25.0))

# auxT layout: [32, W] f32 (transposed constants; 32 DMA packets)
CCT_OFF = 0      # rows 0:6, cols 0:115: ccT (c1m, c1a, c2m, c2a, K, blockid)
EYE_OFF = 115    # cols 115:147: eye(32) (top-left 6x6 doubles as eye(6))
U345_OFF = 147   # cols 147:179: [U3;U4;U5] rows 0:24, zero-padded to 32
U012T_OFF = 179  # cols 179:294: [U0;U1;U2] block-stacked, transposed [32,115]
AUX_W = 294

MM_DT = BF16


def _cct_table() -> np.ndarray:
    """[6, 115] per-partition constants (transposed).

    s1' = rint(v*c1m + c1a);  s2 = rint(v*c2m + c2a)  (rows 0:50: min(v,1))
    onehot[p] = (K[p]*s2 == s1')
    Integer offsets commute with round-to-nearest, so folding the one-hot
    row offset into c1a keeps the floors exact.
    """
    cc = np.zeros((6, KV), np.float32)
    p = np.arange(KV, dtype=np.float32)
    # block 0 (rows 0:50): s1' = floor(v/1000) + 1000 - p ; s2 = min(v,1)
    cc[0, 0:50] = R1000
    cc[1, 0:50] = np.float32(-499.5 * R1000) + 1000.0 - p[0:50]
    cc[4, 0:50] = 1000.0
    # block 1 (rows 50:90): s1' = floor(v/25) + 50 - p ; s2 = floor(v/1000)
    cc[0, 50:90] = R25
    cc[1, 50:90] = np.float32(-12.0 * R25) + 50.0 - p[50:90]
    cc[2, 50:90] = R1000
    cc[3, 50:90] = np.float32(-499.5 * R1000)
    cc[4, 50:90] = 40.0
    cc[5, 50:90] = 1.0
    # block 2 (rows 90:115): s1' = v - 25000 + 90 - p ; s2 = floor(v/25) - 1000
    cc[0, 90:115] = 1.0
    cc[1, 90:115] = -25000.0 + 90.0 - p[90:115]
    cc[2, 90:115] = R25
    cc[3, 90:115] = np.float32(-25012.0 * R25)
    cc[4, 90:115] = 25.0
    cc[5, 90:115] = 2.0
    return cc


def _aux_table(us: list[np.ndarray]) -> np.ndarray:
    aux = np.zeros((32, AUX_W), np.float32)
    aux[0:6, CCT_OFF:CCT_OFF + KV] = _cct_table()
    aux[:, EYE_OFF:EYE_OFF + 32] = np.eye(32, dtype=np.float32)
    # u345 rows are the factor matrices stacked (24 x 32), zero-padded to 32
    aux[0:8, U345_OFF:U345_OFF + 32] = us[3]
    aux[8:16, U345_OFF:U345_OFF + 32] = us[4]
    aux[16:24, U345_OFF:U345_OFF + 32] = us[5]
    # U012T[r, p]: block-stacked factor rows, transposed
    u012 = np.zeros((KV, RANK), np.float32)
    u012[0:50] = us[0]
    u012[50:90] = us[1]
    u012[90:115] = us[2]
    aux[:, U012T_OFF:U012T_OFF + KV] = u012.T
    return aux


def build():
    nc = bacc.Bacc("TRN2", target_bir_lowering=False, debug=False)

    # x int32 narrowed to u16 host-side (pure byte selection; values < 2^16),
    # replicated 4x so the partition-broadcast DMAs read distinct sources and
    # engage more DMA engines (a single-source broadcast pins to ~5 of 16).
    x = nc.dram_tensor("x", [4, PER_CORE], U16, kind="ExternalInput")
    aux_d = nc.dram_tensor("aux", [32, AUX_W], F32, kind="ExternalInput")
    out = nc.dram_tensor("out", [PER_CORE, EMB], BF16, kind="ExternalOutput")

    NH = PER_CORE // HALF   # 2 halves
    NG = PER_CORE // 256    # 4 output groups of 256 rows

    with tile.TileContext(nc) as tc:
        with (
            tc.tile_pool(name="const", bufs=1) as cpool,
            tc.tile_pool(name="work", bufs=2) as wpool,
            # bufs=1: half B's writes must wait for half A's readers, which
            # keeps the scheduler from starving half A's downstream ops
            tc.tile_pool(name="ser", bufs=1) as spool,
            tc.tile_pool(name="gps", bufs=2, space="PSUM") as gpsum,
            tc.tile_pool(name="tps", bufs=1, space="PSUM") as tpsum,
            tc.tile_pool(name="ops", bufs=2, space="PSUM") as opsum,
            tc.tile_pool(name="osb", bufs=4) as opool,
        ):
            # ---- input DMAs: the x broadcast is split into 4 row-blocks from
            # 4 replicas across both HWDGE queues (115 2KB packets total);
            # auxT is 32 packets on the scalar queue.
            auxs = cpool.tile([32, AUX_W], F32)
            nc.scalar.dma_start(out=auxs[:], in_=aux_d[:])
            xrep = cpool.tile([KV, PER_CORE], U16)
            nc.sync.dma_start(
                out=xrep[:], in_=x[0].unsqueeze(0).partition_broadcast(KV)
            )

            eye32 = auxs[:, EYE_OFF:EYE_OFF + 32]

            # iota block-ids [115, 3x32] (no input deps; fires immediately)
            iot = cpool.tile([KV, 3 * RANK], I16)
            nc.gpsimd.iota(
                out=iot[:].rearrange("p (b c) -> p b c", b=3),
                pattern=[[1, 3], [0, RANK]], base=0, channel_multiplier=0,
            )

            # ---- restore per-partition constants via PE transposes
            cc_ps = tpsum.tile([KV, 6], F32)
            nc.tensor.transpose(
                cc_ps[:], auxs[0:6, CCT_OFF:CCT_OFF + KV], eye32[0:6, 0:6]
            )
            cc = cpool.tile([KV, 6], F32)
            nc.scalar.copy(out=cc[:], in_=cc_ps[:])

            u012_ps = tpsum.tile([KV, RANK], F32)
            nc.tensor.transpose(
                u012_ps[:], auxs[:, U012T_OFF:U012T_OFF + KV], eye32
            )
            # block-diag [U0;U1;U2] in bf16: (blockid == iota_b) * U012
            ublk = cpool.tile([KV, 3 * RANK], MM_DT)
            nc.vector.scalar_tensor_tensor(
                out=ublk[:].rearrange("p (b c) -> p b c", b=3),
                in0=iot[:].rearrange("p (b c) -> p b c", b=3),
                scalar=cc[:, 5:6],
                in1=u012_ps[:].unsqueeze(1).broadcast_to([KV, 3, RANK]),
                op0=ALU.is_equal, op1=ALU.mult,
            )

            # u345t via the DVE 32x32 stream transpose (no PSUM round-trip)
            u345t = cpool.tile([RANK, 32], F32)
            nc.vector.transpose(
                out=u345t[:], in_=auxs[:, U345_OFF:U345_OFF + 32]
            )

            # ---- W[r, e] = U3[d,r] * U4[e2,r] * U5[f,r] on GpSimd
            t45 = cpool.tile([RANK, 64], F32)
            nc.gpsimd.tensor_tensor(
                out=t45[:].rearrange("r (e f) -> r e f", e=8),
                in0=u345t[:, 8:16].unsqueeze(2).broadcast_to([RANK, 8, 8]),
                in1=u345t[:, 16:24].unsqueeze(1).broadcast_to([RANK, 8, 8]),
                op=ALU.mult,
            )
            wt = cpool.tile([RANK, EMB], MM_DT)
            nc.gpsimd.tensor_tensor(
                out=wt[:].rearrange("r (d ef) -> r d ef", d=8),
                in0=u345t[:, 0:8].unsqueeze(2).broadcast_to([RANK, 8, 64]),
                in1=t45[:].unsqueeze(1).broadcast_to([RANK, 8, 64]),
                op=ALU.mult,
            )

            vth = [None] * NH
            for h in range(NH):
                xh = xrep[:, h * HALF:(h + 1) * HALF]
                # fused decomposition: 3 tensor_scalar + 1 scalar_tensor_tensor
                s1 = wpool.tile([KV, HALF], I16, name=f"s1_{h}", tag="s1")
                nc.vector.tensor_scalar(
                    out=s1[:], in0=xh, scalar1=cc[:, 0:1], scalar2=cc[:, 1:2],
                    op0=ALU.mult, op1=ALU.add,
                )
                s2 = wpool.tile([KV, HALF], I16, name=f"s2_{h}", tag="s2")
                nc.vector.tensor_scalar(
                    out=s2[:], in0=xh, scalar1=cc[:, 2:3], scalar2=cc[:, 3:4],
                    op0=ALU.mult, op1=ALU.add,
                )
                # rows 0:50: s2 = min(v, 1) folds the v==0 padding mask into
                # the block-0 one-hot (partition starts must be 32-aligned,
                # so overwrite rather than split at row 50)
                nc.vector.tensor_scalar(
                    out=s2[0:50, :], in0=xh[0:50, :], scalar1=1.0, scalar2=1.0,
                    op0=ALU.min, op1=ALU.mult,
                )
                onehot = wpool.tile([KV, HALF], MM_DT, name=f"oh_{h}", tag="oh")
                nc.vector.scalar_tensor_tensor(
                    out=onehot[:], in0=s2[:], scalar=cc[:, 4:5], in1=s1[:],
                    op0=ALU.mult, op1=ALU.is_equal,
                )

                pv = gpsum.tile([3 * RANK, HALF], F32, name=f"pv_{h}", tag="pv")
                nc.tensor.matmul(
                    pv[:], lhsT=ublk[:], rhs=onehot[:], start=True, stop=True,
                )
                # Two SBUF inputs must share a base partition, PSUM inputs
                # are exempt: stage block 0 to SBUF on the Scalar engine,
                # keep blocks 1/2 as the PSUM operand of each multiply.
                s0 = spool.tile([RANK, HALF], F32, name=f"s0_{h}", tag="s0")
                nc.scalar.copy(out=s0[:], in_=pv[0:32, :])
                v01 = spool.tile([RANK, HALF], F32, name=f"v01_{h}", tag="v01")
                nc.vector.tensor_tensor(
                    out=v01[:], in0=s0[:], in1=pv[32:64, :], op=ALU.mult
                )
                vth[h] = cpool.tile([RANK, HALF], MM_DT, name=f"vt_{h}")
                nc.vector.tensor_tensor(
                    out=vth[h][:], in0=v01[:], in1=pv[64:96, :], op=ALU.mult
                )

            # ---- output: per 256-row group, even/odd interleaved columns so
            # partition p holds DRAM rows {r0+2p, r0+2p+1} -> one contiguous
            # 2KB packet per partition per group.  The last group's cast and
            # DMA are split across both engines/partition halves to shorten
            # the tail.
            for g in range(NG):
                h, lo = g // 2, (g % 2) * 256
                vsel = vth[h][:].rearrange("r (n two) -> r n two", two=2)
                po = opsum.tile([128, 2 * EMB], F32, name=f"po_{g}", tag="po")
                for j in range(2):
                    nc.tensor.matmul(
                        po[:, j * EMB:(j + 1) * EMB],
                        lhsT=vsel[:, lo // 2:lo // 2 + 128, j],
                        rhs=wt[:], start=True, stop=True,
                    )
                osb = opool.tile([128, 2 * EMB], MM_DT, name=f"osb_{g}",
                                 tag="osb")
                row0 = g * 256
                oview = out[row0:row0 + 256, :].rearrange(
                    "(p j) e -> p (j e)", j=2
                )
                if g % 2 == 0:
                    nc.scalar.copy(out=osb[:], in_=po[:])
                else:
                    nc.vector.tensor_copy(out=osb[:], in_=po[:])
                nc.sync.dma_start(out=oview, in_=osb[:])

    nc.compile()
    return nc


_CACHE: dict = {}


def _get_nc():
    if "nc" not in _CACHE:
        _CACHE["nc"] = build()
    return _CACHE["nc"]


def run(inputs, **spmd_kwargs):
    nc = _get_nc()
    x = np.ascontiguousarray(inputs["x"].reshape(-1), dtype=np.int32)
    # little-endian u16 view, even offsets hold the values (pure byte pick)
    xu = np.ascontiguousarray(x.view(np.uint16)[0::2])
    us = [
        np.ascontiguousarray(inputs[f"U{j}"], dtype=np.float32) for j in range(6)
    ]
    aux = _aux_table(us)
    in_maps = []
    for i in range(N_CORES):
        xc = xu[i * PER_CORE:(i + 1) * PER_CORE]
        x4 = np.ascontiguousarray(np.broadcast_to(xc, (4, PER_CORE)))
        in_maps.append({"x": x4, "aux": aux})
    res = run_bass_kernel_spmd(
        nc, in_maps, core_ids=list(range(N_CORES)), **spmd_kwargs
    )
    shards = [
        np.asarray(res.results[i]["out"]).astype(np.float32)
        for i in range(N_CORES)
    ]
    full = np.concatenate(shards, axis=0).reshape(4, 2048, EMB)
    return full, res


def kernel(**inputs) -> np.ndarray:
    return run(inputs)[0]
